# revision 50
# baseline (speedup 1.0000x reference)
"""CMC-V2 loss kernel for 8 Trainium2 NeuronCores (Bass/Tile), v2.

Math
----
Same decomposition as before: 9 NT-Xent terms reduce to
  per-pair loss = 5 + (1/4096) sum_i log(S_i - 1) - (10/4096) sum_i cos_i,
  S_i = sum_j exp(5*cos_ij - 5)   (self term included, the -1 removes it),
plus 12 cosine-embedding terms (1 - mean cos).  Constant 9*5 + 12 = 57.

v2 exploits that exp(sim) is SYMMETRIC and that the 9 pairs share
sub-Grams: the N x N pair matrix splits into per-half "diag" sub-Grams
(AA: 12 of them, shared across pairs) and "rect" sub-Grams (AB: 9).
Each [128,128] block is computed ONCE; its row sums feed S for its rows
and its column sums (ones-matmul over exp, partition reduction) feed S
for its columns.  Per core: diag = 2 row-tiles x 9 col-tiles (offsets
0..8 in rolled space; colsums for offsets 1..7), rect = 2 row-tiles x 16
col-tiles (colsums for all).  504 blocks/core vs 1152 in v1.
S is assembled on the HOST from per-core row/col partial sums (log and
final reduction are O(9*4096) host work, like the existing combine).

Build
-----
Host sends roll(X,-256c).T as [1024,2048] fp8e4m3 (layout+dtype prep).
Per half: squares (fp8, ACT/DVE split) -> norm matmul with an all-ones
fp8 [128,P=128] lhsT so the column norms land REPLICATED across all
partitions in PSUM -> ACT Ln -> ACT Exp(-0.5*ln + ln16) gives the
broadcast rinv tile [128,2048] bf16 directly (no reshapes, no DRAM
roundtrips, no broadcast matmul) -> one in-place DVE multiply makes
znt = (16*Zn)^T in fp8.  Dots run on GpSimd.
"""

import numpy as np
import ml_dtypes
from contextlib import ExitStack

from concourse import bass, bacc, tile, mybir
from concourse.bass_utils import run_bass_kernel_spmd

BF16 = mybir.dt.bfloat16
FP8 = mybir.dt.float8e4
F32 = mybir.dt.float32
AF = mybir.ActivationFunctionType
ALU = mybir.AluOpType

FP8_SCALE = 16.0
LN_SCALE = float(np.log(FP8_SCALE))
EXP_SCALE = 5.0 / (FP8_SCALE * FP8_SCALE)
DSCALE = 1.0 / (FP8_SCALE * FP8_SCALE)

B = 2048          # batch
DH = 512          # half feature dim
N_CORES = 8
R = B // N_CORES  # 256 rows per core shard
KC = DH // 128    # 4 contraction chunks per half
NT = B // 128     # 16 col tiles per matrix

# half ids: 2*f + h  (h=0 shared, h=1 private)
NAMES = ["f1_m0", "f1_m1", "f1_m2", "f2_m0", "f2_m1", "f2_m2"]
PAIRS = [(0, 2), (0, 4), (2, 4),      # shared view1
         (6, 8), (6, 10), (8, 10),    # shared view2
         (1, 7), (3, 9), (5, 11)]     # private cross-view
ORTHO = [(0, 1), (2, 3), (4, 5), (1, 3), (1, 5), (3, 5),
         (6, 7), (8, 9), (10, 11), (7, 9), (7, 11), (9, 11)]

N_RSAB = 9 * 2 * 2 + 12 * 2 * 2   # rect (sub,t,2 chunks) + diag (fh,t,A/B)


def build_program(repeat=1, loads_on="gpsimd", timing_mode="full",
                  sq_pat="A", dots_on="vector", gram_bufs=2, lag=0,
                  bbufs=2, mult_mode="inplace", cs_drain="vector",
                  norm_path="replicated"):
    # Restrict ACT table selection to the set with exp AND ln (square is in
    # every set); avoids ~2.7us table reloads on ScalarE.
    if not getattr(bacc, "_ant_act_tables_patched", False):
        _orig_tables = bacc.get_activation_tables

        def _patched(arch):
            tabs = _orig_tables(arch)
            return {k: (v if k == "natural_log_exp_and_others" else set())
                    for k, v in tabs.items()}

        bacc.get_activation_tables = _patched
        bacc._ant_act_tables_patched = True

    nc = bacc.Bacc(
        "TRN2",
        target_bir_lowering=False,
        debug=False,
        enable_asserts=False,
        num_devices=N_CORES,
    )
    ffs = [nc.dram_tensor(n, [2 * DH, B], FP8, kind="ExternalInput").ap()
           for n in NAMES]
    out_part = nc.dram_tensor("part", [128, 4], F32, kind="ExternalOutput").ap()
    out_rsab = nc.dram_tensor("rsab", [128, N_RSAB], F32,
                              kind="ExternalOutput").ap()
    # rect colsums: per sub 4 chunks of 512 (rolled cols 0..2047, DR-stacked
    # over both row-tiles); diag: per (fh, t) 2 chunks of 512 covering local
    # cols 128t+128 .. 128t+1151
    out_csr = nc.dram_tensor("csr", [9, 4, 512], BF16,
                             kind="ExternalOutput").ap()
    out_csd = nc.dram_tensor("csd", [12, 2, 2, 512], BF16,
                             kind="ExternalOutput").ap()

    with tile.TileContext(nc) as tc, ExitStack() as ctx:
        znt_pool = ctx.enter_context(tc.tile_pool(name="zntp", bufs=12))
        raw_pool = ctx.enter_context(tc.tile_pool(name="rawp", bufs=2))
        sq_pool = ctx.enter_context(tc.tile_pool(name="sqp", bufs=bbufs))
        zb_pool = ctx.enter_context(tc.tile_pool(name="zbp", bufs=2))
        bc_pool = ctx.enter_context(tc.tile_pool(name="bcp", bufs=bbufs))
        lgn_pool = ctx.enter_context(tc.tile_pool(name="lgnp", bufs=bbufs))
        es_pool = ctx.enter_context(tc.tile_pool(name="esp", bufs=3))
        cs_pool = ctx.enter_context(tc.tile_pool(name="csp", bufs=4))
        dsc_pool = ctx.enter_context(tc.tile_pool(name="dscp", bufs=2))
        acc_pool = ctx.enter_context(tc.tile_pool(name="accp", bufs=1))
        psum_pool = ctx.enter_context(
            tc.tile_pool(name="psump", bufs=2, space="PSUM"))

        load_engs = {"gpsimd": [nc.gpsimd], "scalar": [nc.scalar],
                     "sync": [nc.sync],
                     "alt": [nc.sync, nc.scalar]}[loads_on]
        dots_eng = {"gpsimd": nc.gpsimd, "vector": nc.vector}[dots_on]

        biasm5 = acc_pool.tile([128, 1], F32, tag="biasm5", name="biasm5")
        nc.gpsimd.memset(biasm5[:], -5.0)
        biasln = acc_pool.tile([128, 1], F32, tag="biasln", name="biasln")
        nc.gpsimd.memset(biasln[:], LN_SCALE)
        # all-ones fp8 weights [K=128, P=128] -> replicated partition sums
        ones128 = acc_pool.tile([128, 128], FP8, tag="ones128", name="ones128")
        nc.gpsimd.memset(ones128[:], 1.0)
        # ones for DR colsums: lhsT [Ki=128, Ko=2, P] (Ko step must be
        # 16B-aligned, so allocate P=16 and slice P=1)
        ones_dr = acc_pool.tile([128, 2, 128], FP8, tag="onesdr",
                                name="ones_dr")
        nc.gpsimd.memset(ones_dr[:], 1.0)
        ones_1 = acc_pool.tile([128, 1], FP8, tag="ones1", name="ones_1")
        nc.gpsimd.memset(ones_1[:], 1.0)
        ones_col = acc_pool.tile([1, 128], BF16, tag="onesc", name="ones_col")
        nc.gpsimd.memset(ones_col[:], 1.0)

        rsab = acc_pool.tile([128, N_RSAB], F32, tag="rsab", name="rsab_sb")
        dots_all = acc_pool.tile([128, 21], F32, tag="dots", name="dots_all")
        part = acc_pool.tile([128, 4], F32, tag="part", name="part_sb")

        znt = {}
        sq_i = [0]
        bstage = 9
        if timing_mode.startswith("builds") and timing_mode[6:].isdigit():
            bstage = int(timing_mode[6:])
        # fixed rsab column layout (host combine depends on it):
        # diag (fh, t): cols 4*fh + 2*t + {0:A, 1:B}; rect (s, t):
        # cols 48 + 4*s + 2*t + {0, 1}
        RECT0 = 48

        def build_ff(fh):
            """Load raw fp8 XT half, normalize in place -> znt = (16*Zn)^T."""
            f, h = fh // 2, fh % 2
            zt = (raw_pool if mult_mode == "dmacast" else znt_pool).tile(
                [128, KC, B], FP8, tag="znt", name=f"znt{fh}")
            load_engs[fh % len(load_engs)].dma_start(
                out=zt[:],
                in_=ffs[f][h * DH:(h + 1) * DH, :].rearrange(
                    "(kc p) n -> p kc n", p=128))
            sq = sq_pool.tile([128, KC, B], FP8, tag="sq", name=f"sq{fh}")
            if bstage < 1:
                znt[fh] = zt
                return
            # squares engine per sq_pat cycle ("A"=ACT, "D"=DVE)
            if sq_pat[sq_i[0] % len(sq_pat)] == "A":
                nc.scalar.activation(sq[:], zt[:], AF.Square)
            else:
                nc.vector.scalar_tensor_tensor(
                    out=sq[:], in0=zt[:], scalar=1.0, in1=zt[:],
                    op0=ALU.mult, op1=ALU.mult)
            sq_i[0] += 1
            bc = bc_pool.tile([128, B], BF16, tag="bc", name=f"bc{fh}")
            if bstage < 2:
                nc.vector.memset(bc[:, 0:2], 1.0)
                znt[fh] = zt
                return
            if norm_path == "small":
                # norms -> [1,B] bf16 row -> DRAM roundtrip -> [128,16] ->
                # tiny Ln/Exp -> [1,B] rinv row -> K=1 broadcast matmul
                nrow = lgn_pool.tile([1, B], BF16, tag="nrow",
                                     name=f"nr{fh}")
                for rnd in range(4):
                    nps = psum_pool.tile([128, 512], F32, tag="normp",
                                         bufs=2, name=f"nps{fh}_{rnd}")
                    cs = slice(rnd * 512, (rnd + 1) * 512)
                    for kc in range(KC):
                        nc.tensor.matmul(nps[0:1, :], ones_1[:],
                                         sq[:, kc, cs],
                                         start=(kc == 0),
                                         stop=(kc == KC - 1))
                    nc.vector.tensor_copy(nrow[:, cs], nps[0:1, :])
                nrow_d = nc.dram_tensor(f"nrow_d{fh}", [B], BF16,
                                        kind="Internal").ap()
                rinv_d = nc.dram_tensor(f"rinv_d{fh}", [128, B // 128],
                                        BF16, kind="Internal").ap()
                nc.sync.dma_start(out=nrow_d.rearrange("(a b) -> a b", a=1),
                                  in_=nrow[:])
                n128 = lgn_pool.tile([128, B // 128], BF16, tag="n128",
                                     name=f"n128_{fh}")
                nc.sync.dma_start(
                    out=n128[:], in_=nrow_d.rearrange("(p c) -> p c", p=128))
                lgn = lgn_pool.tile([128, B // 128], F32, tag="lgn",
                                    name=f"lgn{fh}")
                nc.scalar.activation(lgn[:], n128[:], AF.Ln)
                rinv = lgn_pool.tile([128, B // 128], BF16, tag="rinv",
                                     name=f"ri{fh}")
                nc.scalar.activation(rinv[:], lgn[:], AF.Exp, scale=-0.5,
                                     bias=biasln[:])
                nc.scalar.dma_start(out=rinv_d, in_=rinv[:])
                rrow = lgn_pool.tile([1, B], BF16, tag="rrow",
                                     name=f"rr{fh}")
                nc.scalar.dma_start(
                    out=rrow[:],
                    in_=rinv_d.rearrange("p c -> (p c)").rearrange(
                        "(a b) -> a b", a=1))
                for cb in range(4):
                    bps = psum_pool.tile([128, 512], F32, tag="normp",
                                         bufs=2, name=f"bps{fh}_{cb}")
                    cs = slice(cb * 512, (cb + 1) * 512)
                    nc.tensor.matmul(bps[:], ones_col[:], rrow[:, cs],
                                     start=True, stop=True)
                    nc.vector.tensor_copy(bc[:, cs], bps[:])
            else:
                for rnd in range(2):   # 2 x [128, 2, 512] PSUM rounds
                    nps = psum_pool.tile([128, 2, 512], F32, tag="normp",
                                         bufs=1, name=f"nps{fh}_{rnd}")
                    for cbl in range(2):
                        cs = slice((2 * rnd + cbl) * 512,
                                   (2 * rnd + cbl + 1) * 512)
                        for kc in range(KC):
                            nc.tensor.matmul(nps[:, cbl, :], ones128[:],
                                             sq[:, kc, cs],
                                             start=(kc == 0),
                                             stop=(kc == KC - 1))
                    lgn = lgn_pool.tile([128, 2, 512], F32, tag="lgn",
                                        name=f"lgn{fh}_{rnd}")
                    nc.scalar.activation(lgn[:], nps[:], AF.Ln)
                    # bc = exp(-0.5*ln(n) + ln16) = 16/sqrt(n), replicated
                    nc.scalar.activation(bc[:, rnd * 1024:(rnd + 1) * 1024],
                                         lgn[:], AF.Exp, scale=-0.5,
                                         bias=biasln[:])
            if bstage < 3:
                znt[fh] = zt
                return
            if mult_mode == "dmacast":
                # bf16-out multiply (fast DVE path) + SWDGE cast to fp8
                zb = zb_pool.tile([128, KC, B], BF16, tag="zb",
                                  name=f"zb{fh}")
                for kc in range(KC):
                    nc.vector.scalar_tensor_tensor(
                        out=zb[:, kc, :], in0=zt[:, kc, :], scalar=1.0,
                        in1=bc[:], op0=ALU.mult, op1=ALU.mult)
                zn = znt_pool.tile([128, KC, B], FP8, tag="znt2",
                                   name=f"znn{fh}")
                nc.gpsimd.dma_start(out=zn[:], in_=zb[:])
                znt[fh] = zn
            elif mult_mode == "off":   # timing probe only (wrong numerics)
                znt[fh] = zt
            else:
                for kc in range(KC):   # in-place: znt = raw * bc
                    nc.vector.scalar_tensor_tensor(
                        out=zt[:, kc, :], in0=zt[:, kc, :], scalar=1.0,
                        in1=bc[:], op0=ALU.mult, op1=ALU.mult)
                znt[fh] = zt

        def diag(fh):
            """Self sub-Gram of half fh: 2 row-tiles x col offsets 0..8."""
            zt = znt[fh]
            for t in range(2):
                base = 128 * t
                rcol = 4 * fh + 2 * t
                es = es_pool.tile([128, 1152], FP8, tag="esd",
                                  name=f"esd{fh}_{t}")
                # chunk A: local cols [base, base+1024) -> offsets 0..7
                psA = psum_pool.tile([128, 2, 512], F32, tag="gram",
                                    bufs=gram_bufs, name=f"dA{fh}_{t}")
                for cbl in range(2):
                    cs = slice(base + cbl * 512, base + (cbl + 1) * 512)
                    for q in range(KC // 2):
                        nc.tensor.matmul(
                            psA[:, cbl, :],
                            zt[:, 2 * q:2 * q + 2, base:base + 128],
                            zt[:, 2 * q:2 * q + 2, cs],
                            perf_mode=mybir.MatmulPerfMode.DoubleRow,
                            start=(q == 0), stop=(q == KC // 2 - 1))
                nc.scalar.activation(
                    es[:, 0:1024].rearrange("p (a b) -> p a b", a=2),
                    psA[:], AF.Exp, bias=biasm5[:], scale=EXP_SCALE,
                    accum_out=rsab[:, rcol:rcol + 1])
                # chunk B: local cols [base+1024, base+1152) -> offset 8
                psB = psum_pool.tile([128, 2, 512], F32, tag="gram",
                                     bufs=gram_bufs, name=f"dB{fh}_{t}")
                for q in range(KC // 2):
                    nc.tensor.matmul(
                        psB[:, 0, 0:128],
                        zt[:, 2 * q:2 * q + 2, base:base + 128],
                        zt[:, 2 * q:2 * q + 2, base + 1024:base + 1152],
                        perf_mode=mybir.MatmulPerfMode.DoubleRow,
                        start=(q == 0), stop=(q == KC // 2 - 1))
                nc.scalar.activation(
                    es[:, 1024:1152], psB[:, 0, 0:128], AF.Exp,
                    bias=biasm5[:], scale=EXP_SCALE,
                    accum_out=rsab[:, rcol + 1:rcol + 2])
                # colsums over local cols [128, 1024): offsets 1..7 only
                # (self tile = rowsum; offset 8 counted by both rowsums)
                for ck, w in ((0, 512), (1, 384)):
                    cps = psum_pool.tile([128, 512], F32, tag="cs", bufs=2,
                                         name=f"dc{fh}_{t}_{ck}")
                    nc.tensor.matmul(cps[:, 0:w], ones128[:],
                                     es[:, 128 + ck * 512:128 + ck * 512 + w],
                                     start=True, stop=True)
                    csb = cs_pool.tile([128, 512], BF16, tag="csb",
                                       name=f"dcs{fh}_{t}_{ck}")
                    nc.vector.tensor_copy(csb[:, 0:w], cps[:, 0:w])
                    nc.sync.dma_start(
                        out=out_csd[fh, t, ck, 0:w].rearrange(
                            "(a b) -> a b", a=1),
                        in_=csb[0:1, 0:w])

        def rect(s):
            """Cross sub-Gram for pair s: rows = shard of a, cols = all of b.
            es kept per (chunk) stacked over both row-tiles for DR colsums."""
            a, b = PAIRS[s]
            za, zb = znt[a], znt[b]
            ess = []
            for t in range(2):
                base = 128 * t
                for half_c in range(2):   # 2 psum tiles of [128, 2, 512]
                    ps = psum_pool.tile([128, 2, 512], F32, tag="gram",
                                        bufs=gram_bufs, name=f"r{s}_{t}_{half_c}")
                    for cbl in range(2):
                        cb = half_c * 2 + cbl
                        for q in range(KC // 2):
                            nc.tensor.matmul(
                                ps[:, cbl, :],
                                za[:, 2 * q:2 * q + 2, base:base + 128],
                                zb[:, 2 * q:2 * q + 2,
                                   cb * 512:(cb + 1) * 512],
                                perf_mode=mybir.MatmulPerfMode.DoubleRow,
                                start=(q == 0), stop=(q == KC // 2 - 1))
                    if t == 0:
                        es = es_pool.tile([128, 2, 2, 512], FP8, tag="esr",
                                          name=f"esr{s}_{half_c}")
                        ess.append(es)
                    else:
                        es = ess[half_c]
                    rcol = RECT0 + 4 * s + 2 * t + half_c
                    nc.scalar.activation(
                        es[:, t, :, :], ps[:], AF.Exp, bias=biasm5[:],
                        scale=EXP_SCALE,
                        accum_out=rsab[:, rcol:rcol + 1])
            # DR colsums: contract over rows of both tiles (K=256)
            for ck in range(4):
                es = ess[ck // 2]
                cps = psum_pool.tile([128, 512], F32, tag="cs", bufs=2,
                                     name=f"rc{s}_{ck}")
                nc.tensor.matmul(
                    cps[:], ones_dr[:],
                    es[:, :, ck % 2, :],
                    perf_mode=mybir.MatmulPerfMode.DoubleRow,
                    start=True, stop=True)
                csb = cs_pool.tile([128, 512], BF16, tag="csb",
                                   name=f"rcs{s}_{ck}")
                nc.vector.tensor_copy(csb[:], cps[:])
                nc.sync.dma_start(
                    out=out_csr[s, ck, :].rearrange("(a b) -> a b", a=1),
                    in_=csb[0:1, :])

        def dots(col, X, Y):
            """dots_all[:, col] = per-partition sum over the 256-row shard
            of <Zn_X[i], Zn_Y[i]>."""
            o = dsc_pool.tile([128, KC, R], F32, tag="dsc", name=f"do{col}")
            dots_eng.scalar_tensor_tensor(
                out=o[:], in0=znt[X][:, :, 0:R], scalar=DSCALE,
                in1=znt[Y][:, :, 0:R], op0=ALU.mult, op1=ALU.mult,
                accum_out=dots_all[:, col:col + 1])

        if timing_mode == "grams":
            shared = []
            for sI in range(9):
                t = znt_pool.tile([128, KC, B], FP8, tag="znt",
                                  name=f"znts{sI}")
                nc.vector.memset(t[:, :, 0:2], 0.0)
                shared.append(t)
            for fh in range(12):
                znt[fh] = shared[fh % 9]
            def build_ff(fh):
                pass
        elif timing_mode.startswith("builds"):
            nc.vector.memset(rsab[:], 1.0)
            def diag(fh):
                pass
            def rect(s):
                pass

        rep_ctx = tc.For_i(0, repeat, 1) if repeat > 1 else None
        if rep_ctx is not None:
            rep_ctx.__enter__()

        if timing_mode == "grams":
            nc.vector.memset(rsab[:], 0.0)

        # Emission: halves in a pair-friendly order; a diag follows its
        # build; rects and dots as soon as both operands exist.  dots cols:
        # 0..8 contrastive (PAIRS order), 9..20 ortho (ORTHO order).
        rect_done = set()
        ortho_done = set()

        def emit_ready():
            for s, (a, b) in enumerate(PAIRS):
                if s not in rect_done and a in znt and b in znt:
                    rect(s)
                    dots(s, a, b)
                    rect_done.add(s)
            for o, (a, b) in enumerate(ORTHO):
                if o not in ortho_done and a in znt and b in znt:
                    dots(9 + o, a, b)
                    ortho_done.add(o)

        order = [0, 2, 4, 6, 8, 10, 1, 7, 3, 9, 5, 11]
        pend = []
        for fh in order:
            build_ff(fh)
            pend.append(fh)
            if len(pend) > lag:
                diag(pend.pop(0))
            emit_ready()
        for fh in pend:
            diag(fh)
            emit_ready()

        # ---- epilogue: dots partial sums ----
        nc.vector.memset(part[:], 0.0)
        nc.vector.tensor_reduce(part[:, 1:2], dots_all[:, 0:9],
                                axis=mybir.AxisListType.X, op=ALU.add)
        nc.vector.tensor_reduce(part[:, 2:3], dots_all[:, 9:21],
                                axis=mybir.AxisListType.X, op=ALU.add)
        nc.sync.dma_start(out=out_part, in_=part[:])
        nc.scalar.dma_start(out=out_rsab, in_=rsab[:])

        if rep_ctx is not None:
            rep_ctx.__exit__(None, None, None)

    nc.compile()
    return nc


_PROG = None


def _get_prog():
    global _PROG
    if _PROG is None:
        _PROG = build_program()
    return _PROG


def make_in_maps(inputs):
    f8 = ml_dtypes.float8_e4m3
    base = {n: np.ascontiguousarray(
                np.asarray(inputs[n], dtype=np.float32).T).astype(f8)
            for n in NAMES}
    in_maps = []
    for c in range(N_CORES):
        m = {}
        for n in NAMES:
            if c == 0:
                m[n] = base[n]
            else:
                m[n] = np.ascontiguousarray(np.roll(base[n], -R * c, axis=1))
        in_maps.append(m)
    return in_maps


def combine(results):
    """results: list of 8 dicts with part/rsab/csr/csd -> scalar loss."""
    S_diag_row = np.zeros((12, B))   # diag rowsums per half, global rows
    S_diag_col = np.zeros((12, B))
    S_rect_row = np.zeros((9, B))    # rows of pair's a-side
    S_rect_col = np.zeros((9, B))    # rows of pair's b-side
    tcc = toc = 0.0

    for c, r in enumerate(results):
        part = np.asarray(r["part"], dtype=np.float64)
        tcc += part[:, 1].sum()
        toc += part[:, 2].sum()
        rsab = np.asarray(r["rsab"], dtype=np.float64)   # [128, N_RSAB]
        csr = np.asarray(r["csr"], dtype=np.float64)     # [9, 4, 512]
        csd = np.asarray(r["csd"], dtype=np.float64)     # [12, 2, 2, 512]
        for fh in range(12):
            for t in range(2):
                col = 4 * fh + 2 * t
                rows = (np.arange(128) + 128 * t + R * c) % B
                S_diag_row[fh, rows] += rsab[:, col] + rsab[:, col + 1]
                # colsums cover local cols 128t+128 .. 128t+1023
                lc = 128 * t + 128 + np.arange(896)
                gc = (lc + R * c) % B
                S_diag_col[fh, gc] += np.concatenate(
                    [csd[fh, t, 0, :], csd[fh, t, 1, 0:384]])
        for s in range(9):
            for t in range(2):
                col = 48 + 4 * s + 2 * t
                rows = (np.arange(128) + 128 * t + R * c) % B
                S_rect_row[s, rows] += rsab[:, col] + rsab[:, col + 1]
            gc = (np.arange(B) + R * c) % B
            S_rect_col[s, gc] += csr[s].reshape(B)

    tl = 0.0
    for s, (a, b) in enumerate(PAIRS):
        Sa = S_diag_row[a] + S_diag_col[a] + S_rect_row[s]
        Sb = S_diag_row[b] + S_diag_col[b] + S_rect_col[s]
        tl += np.log(Sa - 1.0).sum() + np.log(Sb - 1.0).sum()

    n2 = float(2 * B)
    loss = (9 * 5.0 + 12.0) + tl / n2 - 10.0 * tcc / n2 - toc / float(B)
    return np.float32(loss)


def kernel(**inputs):
    nc = _get_prog()
    in_maps = make_in_maps(inputs)
    res = run_bass_kernel_spmd(nc, in_maps, list(range(N_CORES)))
    return combine([res.results[c] for c in range(N_CORES)])
